# revision 1
# baseline (speedup 1.0000x reference)
"""EHM (SMPLX body + FLAME head + MANO hands) Bass kernel for 8 TRN2 NeuronCores.

Sharding: VERTEX sharding — model weights dominate HBM traffic, so each core
owns 1/8 of the SMPLX vertices (plus the FLAME/MANO vertices its SMPLX rows
stitch in) and computes ALL B=128 batch elements for its shard.

v2: no cross-core communication at all.  Joint regression is folded into the
host-precomputed weight matrix Jmat = J_regressor @ [shapedirs | template], so
joints come from a tiny on-device matmul over betas (pure weight transform,
batch-independent).  FK runs on Vector+GpSimd concurrently with the blend-shape
matmuls on Tensor.  Skinning uses fp16 SBUF operands for 2x DVE throughput.

Per-vertex data layout: [vertex(partition<=128), (c, b)] with c-major free dim
(col = c*128 + b).  Batch-staged data (poses, FK, A matrices): [b(part), free].
"""

import sys

sys.path.insert(0, "/opt/trn_rl_repo")

from contextlib import ExitStack

import numpy as np
import ml_dtypes

BF16NP = ml_dtypes.bfloat16
F16NP = np.float16

import concourse.bass as bass
import concourse.bacc as bacc
import concourse.tile as tile
import concourse.mybir as mybir
from concourse.bass_utils import run_bass_kernel_spmd

F32 = mybir.dt.float32
BF16 = mybir.dt.bfloat16
F16 = mybir.dt.float16
AF = mybir.ActivationFunctionType
ALU = mybir.AluOpType

# ---------------------------------------------------------------- constants
B = 128
VS, VF, VM = 10475, 5023, 778
NL = 350
NCORES = 8

SMPLX_PARENTS = np.array([-1,0,0,0,1,2,3,4,5,6,7,8,9,9,9,12,13,14,16,17,18,19,
                          15,15,15,20,25,26,20,28,29,20,31,32,20,34,35,20,37,38,
                          21,40,41,21,43,44,21,46,47,21,49,50,21,52,53])
FLAME_PARENTS = np.array([-1,0,1,1,1])
MANO_PARENTS = np.array([-1,0,1,2,0,4,5,0,7,8,0,10,11,0,13,14])

N_PLAIN, N_HEAD, N_HL, N_HR = 768, 384, 128, 128
ROWS = N_PLAIN + N_HEAD + N_HL + N_HR        # 1408
NCH = ROWS // 128                            # 11
NCH_PLAIN = 6
CH_PLAIN = set(range(0, NCH_PLAIN))
CH_HEAD0 = 6                                 # chunks 6,7,8 head; 9 L; 10 R
CH_HL, CH_HR = 9, 10

PD_S_K = 189
PD_F_K = 27
PD_M_K = 135

NJ_ALL = 92
OFF_S, OFF_F, OFF_L, OFF_R = 0, 55, 60, 76
NROT = 55
ROT_S0, ROT_F0, ROT_L0, ROT_R0 = 0, 22, 25, 40

BF16_INPUTS = {"sd_s", "pd_s_a", "pd_s_b", "sd_f", "pd_f",
               "sd_m", "pd_m_a", "pd_m_b", "cb16"}
F16_INPUTS = {"w_s", "wre_f", "w_m"}


def _fk_forest():
    par = np.empty(NJ_ALL, np.int64)
    par[OFF_S:OFF_S + 55] = SMPLX_PARENTS
    par[OFF_F:OFF_F + 5] = np.where(FLAME_PARENTS < 0, -1, FLAME_PARENTS + OFF_F)
    par[OFF_L:OFF_L + 16] = np.where(MANO_PARENTS < 0, -1, MANO_PARENTS + OFF_L)
    par[OFF_R:OFF_R + 16] = np.where(MANO_PARENTS < 0, -1, MANO_PARENTS + OFF_R)
    return par


def _fk_levels(par, joints=None):
    """Levels of (j0, dj, n, p0, dp) runs, restricted to `joints` (iterable)."""
    depth = np.zeros(NJ_ALL, np.int64)
    for j in range(NJ_ALL):
        if par[j] >= 0:
            depth[j] = depth[par[j]] + 1
    sel = np.zeros(NJ_ALL, bool)
    sel[np.asarray(list(joints) if joints is not None else range(NJ_ALL))] = True
    levels = []
    for d in range(1, int(depth.max()) + 1):
        js = np.nonzero((depth == d) & sel)[0]
        runs, i = [], 0
        while i < len(js):
            j0, p0 = int(js[i]), int(par[js[i]])
            if i + 1 < len(js):
                ds = int(js[i + 1]) - j0
                ps = int(par[js[i + 1]]) - p0
            else:
                ds, ps = 1, 0
            n = 1
            while (i + n < len(js) and int(js[i + n]) == j0 + n * ds
                   and int(par[js[i + n]]) == p0 + n * ps):
                n += 1
            if n == 1:
                ds, ps = 1, 0
            runs.append((j0, ds, n, p0, ps))
            i += n
        if runs:
            levels.append((d, runs))
    return levels


# ================================================================ host prep

def _split_sizes(total, parts):
    q, r = divmod(total, parts)
    return [q + (1 if i < r else 0) for i in range(parts)]


def _pad_ids(ids, n):
    out = np.full(n, -1, np.int64)
    out[:len(ids)] = ids
    return out


def _host_prep(inp):
    f32 = np.float32
    s2f = np.asarray(inp["smplx2flame_ind"])
    head_ix = np.asarray(inp["head_index"])
    s2l = np.asarray(inp["smplx2mano_left"])
    s2r = np.asarray(inp["smplx2mano_right"])

    head_sv = s2f[head_ix]
    special = np.zeros(VS, bool)
    special[head_sv] = True
    special[s2l] = True
    special[s2r] = True
    plain_sv = np.nonzero(~special)[0]

    pl_sp = np.cumsum([0] + _split_sizes(len(plain_sv), NCORES))
    hd_sp = np.cumsum([0] + _split_sizes(len(head_ix), NCORES))
    hl_sp = np.cumsum([0] + _split_sizes(VM, NCORES))

    sd_s_np = np.asarray(inp["smplx_shapedirs"], f32)
    pd_s_np = np.asarray(inp["smplx_posedirs"], f32)
    jr_s_np = np.asarray(inp["smplx_J_regressor"], f32)
    w_s_np = np.asarray(inp["smplx_lbs_weights"], f32)
    tmpl_s = np.asarray(inp["smplx_v_template"], f32)
    sd_f_np = np.asarray(inp["flame_shapedirs"], f32)
    pd_f_np = np.asarray(inp["flame_posedirs"], f32)
    jr_f_np = np.asarray(inp["flame_J_regressor"], f32)
    w_f_np = np.asarray(inp["flame_lbs_weights"], f32)
    tmpl_f = np.asarray(inp["flame_v_template"], f32)
    re_np = np.asarray(inp["r_eyelid"], f32)
    le_np = np.asarray(inp["l_eyelid"], f32)
    sd_m_np = np.asarray(inp["mano_shapedirs"], f32)
    pd_m_np = np.asarray(inp["mano_posedirs"], f32)
    jr_m_np = np.asarray(inp["mano_J_regressor"], f32)
    w_m_np = np.asarray(inp["mano_lbs_weights"], f32)
    tmpl_m = np.asarray(inp["mano_v_template"], f32)

    aa = np.concatenate([
        np.asarray(inp["global_pose"], f32).reshape(B, 3),
        np.asarray(inp["body_pose"], f32).reshape(B, 63),
        np.asarray(inp["jaw_params"], f32).reshape(B, 3),
        np.asarray(inp["eye_pose"], f32).reshape(B, 6),
        np.asarray(inp["left_hand_pose"], f32).reshape(B, 45),
        np.asarray(inp["right_hand_pose"], f32).reshape(B, 45),
    ], axis=1)

    ep = np.asarray(inp["eyelid_params"], f32)
    aux = np.concatenate([
        np.asarray(inp["head_scale"], f32)[:, None],
        np.asarray(inp["left_hand_scale"], f32)[:, None],
        np.asarray(inp["right_hand_scale"], f32)[:, None],
        ep[:, 0:1], ep[:, 1:2],
        np.asarray(inp["head_pos_offset"], f32),
        np.asarray(inp["left_hand_pos_offset"], f32),
        np.asarray(inp["right_hand_pos_offset"], f32),
    ], axis=1)                                               # [128, 14]

    def beta_T(second):
        b = np.concatenate([np.asarray(inp["shape_params"], f32), second], 1)
        bt = np.zeros((384, B), f32)
        bt[:NL] = b.T
        bt[NL] = 1.0
        return bt.reshape(3, 128, B)

    betaT_s = beta_T(np.asarray(inp["body_exp"], f32))
    betaT_f = beta_T(np.asarray(inp["flame_exp"], f32))

    joff = np.asarray(inp["joints_offset"], f32)

    # ---- J regression folded into weights: Jmat = Jreg @ [shapedirs|tmpl] ----
    def jdirs(jr, sd, tmpl, nj):
        ext = np.concatenate([sd, tmpl[:, :, None]], axis=2)      # [V,3,L+1]
        jm = jr @ ext.reshape(ext.shape[0], -1)                   # [nj, 3*(L+1)]
        return jm.reshape(nj, 3, ext.shape[2])

    jm_s = jdirs(jr_s_np, sd_s_np, tmpl_s, 55)                    # [55,3,351]
    jm_f = jdirs(jr_f_np, sd_f_np, tmpl_f, 5)                     # [5,3,351]
    jm_m = jdirs(jr_m_np, sd_m_np, tmpl_m, 16)                    # [16,3,11]

    def reld(jm, par):
        r = jm.copy()
        r[1:] -= jm[par[1:]]
        return r

    jmrel_s = reld(jm_s, SMPLX_PARENTS)
    jmrel_f = reld(jm_f, FLAME_PARENTS)

    # device lhsT layout: jd[lk, l, c*nj + j] = jm[j, c, lk*128 + l]
    def jd_pack(jm, nj):
        out = np.zeros((3, 128, 3 * nj), f32)
        L = jm.shape[2]
        for lk in range(3):
            l0, l1 = lk * 128, min((lk + 1) * 128, L)
            blk = jm[:, :, l0:l1]                                 # [nj,3,n]
            out[lk, :l1 - l0] = blk.transpose(2, 1, 0).reshape(l1 - l0, 3 * nj)
        return out

    jd_s = jd_pack(jm_s, 55)
    jdr_s = jd_pack(jmrel_s, 55)
    jd_f = jd_pack(jm_f, 5)
    jdr_f = jd_pack(jmrel_f, 5)

    # MANO joints are batch-independent (shared betas): compute on host.
    bm_ext = np.concatenate([np.asarray(inp["mano_betas"], f32)[0], [1.0]])
    jmano = np.einsum('jcl,l->jc', jm_m, bm_ext)                  # [16,3]
    relmano = jmano.copy()
    relmano[1:] -= jmano[MANO_PARENTS[1:]]
    jmb = np.broadcast_to(jmano.T.reshape(1, 48), (B, 48)).copy()
    relmb = np.broadcast_to(relmano.T.reshape(1, 48), (B, 48)).copy()

    betam = np.zeros((11, 1), f32)
    betam[:10, 0] = np.asarray(inp["mano_betas"], f32)[0]
    betam[10, 0] = 1.0

    # joints_offset in batch layout + its parent-relative differences
    joffb = np.ascontiguousarray(joff.transpose(0, 2, 1)).reshape(B, 165)
    joffr = joff.copy()
    joffr[:, 1:] -= joff[:, SMPLX_PARENTS[1:]]
    joffrelb = np.ascontiguousarray(joffr.transpose(0, 2, 1)).reshape(B, 165)

    # pack all small constants into two staging buffers (2 DMAs, not ~20)
    cb32 = np.zeros((128, 763), f32)
    cb32[:, 0:128] = np.eye(128, dtype=f32)
    cb32[:, 128:293] = aa
    cb32[:, 293:307] = aux
    cb32[:, 307:355] = jmb
    cb32[:, 355:403] = relmb
    cb32[:, 403:568] = joffb
    cb32[:, 568:733] = joffrelb
    cb16 = np.zeros((128, 1849), f32)
    cb16[:, 0:384] = betaT_s.transpose(1, 0, 2).reshape(128, 384)
    cb16[:, 384:768] = betaT_f.transpose(1, 0, 2).reshape(128, 384)
    cb16[:, 768:1263] = jd_s.transpose(1, 0, 2).reshape(128, 495)
    cb16[:, 1263:1758] = jdr_s.transpose(1, 0, 2).reshape(128, 495)
    cb16[:, 1758:1803] = jd_f.transpose(1, 0, 2).reshape(128, 45)
    cb16[:, 1803:1848] = jdr_f.transpose(1, 0, 2).reshape(128, 45)
    cb16[0:11, 1848] = betam[:, 0]
    rep = dict(cb32=cb32, cb16=cb16)

    in_maps = []
    vid_all = np.full((NCORES, ROWS), -1, np.int64)

    for c in range(NCORES):
        p_ids = plain_sv[pl_sp[c]:pl_sp[c + 1]]
        h_pos = np.arange(hd_sp[c], hd_sp[c + 1])
        h_sv, h_fv = head_sv[h_pos], head_ix[h_pos]
        l_pos = np.arange(hl_sp[c], hl_sp[c + 1])
        r_pos = l_pos                                         # same split for R
        l_sv, r_sv = s2l[l_pos], s2r[r_pos]

        vid = np.full(ROWS, -1, np.int64)
        vid[:len(p_ids)] = p_ids
        vid[N_PLAIN:N_PLAIN + len(h_sv)] = h_sv
        vid[N_PLAIN + N_HEAD:N_PLAIN + N_HEAD + len(l_sv)] = l_sv
        vid[N_PLAIN + N_HEAD + N_HL:N_PLAIN + N_HEAD + N_HL + len(r_sv)] = r_sv
        vid_all[c] = vid
        vok = vid >= 0
        vc = np.where(vok, vid, 0)

        # smplx shapedirs slab, PLAIN chunks only: [6, 128(p=l), (c, lk, v)]
        pvc = vc[:N_PLAIN]
        pvok = vok[:N_PLAIN]
        sdp = np.zeros((N_PLAIN, 3, 384), f32)
        sdp[:, :, :NL] = np.where(pvok[:, None, None], sd_s_np[pvc], 0.0)
        sdp[:, :, NL] = np.where(pvok[:, None], tmpl_s[pvc], 0.0)
        slab = sdp.reshape(NCH_PLAIN, 128, 3, 3, 128).transpose(0, 4, 2, 3, 1)
        sd_s = np.ascontiguousarray(slab).reshape(NCH_PLAIN, 128, 1152)

        # smplx posedirs, all chunks
        colv = vc[:, None] * 3 + np.arange(3)[None, :]
        pdv = pd_s_np[:PD_S_K][:, colv]
        pdv = np.where(vok[None, :, None], pdv, 0.0)
        pdv = pdv.reshape(PD_S_K, NCH, 128, 3).transpose(1, 0, 3, 2)
        pd_s_a = np.ascontiguousarray(pdv[:, :128]).reshape(NCH, 128, 384)
        pd_s_b = np.ascontiguousarray(pdv[:, 128:]).reshape(NCH, PD_S_K - 128, 384)

        w_s = np.ascontiguousarray(
            np.where(vok[:, None], w_s_np[vc], 0.0)
            .reshape(NCH, 128, 55).transpose(0, 2, 1))

        # flame: 3 gathered head chunks only
        fg = _pad_ids(h_fv, N_HEAD)
        fok = fg >= 0
        fc = np.where(fok, fg, 0)
        sdfp = np.zeros((N_HEAD, 3, 384), f32)
        sdfp[:, :, :NL] = np.where(fok[:, None, None], sd_f_np[fc], 0.0)
        sdfp[:, :, NL] = np.where(fok[:, None], tmpl_f[fc], 0.0)
        slab = sdfp.reshape(3, 128, 3, 3, 128).transpose(0, 4, 2, 3, 1)
        sd_f = np.ascontiguousarray(slab).reshape(3, 128, 1152)

        colf = fc[:, None] * 3 + np.arange(3)[None, :]
        pdfv = pd_f_np[9:36][:, colf]
        pdfv = np.where(fok[None, :, None], pdfv, 0.0)
        pdfv = pdfv.reshape(PD_F_K, 3, 128, 3).transpose(1, 0, 3, 2)
        pd_f = np.ascontiguousarray(pdfv).reshape(3, PD_F_K, 384)

        wre = np.zeros((3, 11, 128), f32)
        for k in range(3):
            rows, ok = fc[k * 128:(k + 1) * 128], fok[k * 128:(k + 1) * 128]
            wre[k, :5] = np.where(ok[None, :], w_f_np[rows].T, 0.0)
            wre[k, 5:8] = np.where(ok[None, :], re_np[rows].T, 0.0)
            wre[k, 8:11] = np.where(ok[None, :], le_np[rows].T, 0.0)

        # mano hands
        m_rows = np.stack([_pad_ids(l_pos, 128), _pad_ids(r_pos, 128)])
        mok = m_rows >= 0
        mc = np.where(mok, m_rows, 0)
        sd_m = np.zeros((2, 11, 384), f32)
        pd_m_a = np.zeros((2, 128, 384), f32)
        pd_m_b = np.zeros((2, PD_M_K - 128, 384), f32)
        w_m = np.zeros((2, 16, 128), f32)
        for h in range(2):
            sdm = np.where(mok[h][:, None, None], sd_m_np[mc[h]], 0.0)
            sd_m[h, :10] = sdm.transpose(2, 1, 0).reshape(10, 384)
            sd_m[h, 10] = np.where(mok[h][:, None], tmpl_m[mc[h]], 0.0).T.reshape(384)
            colm = mc[h][:, None] * 3 + np.arange(3)[None, :]
            pdm = pd_m_np[:, colm]
            pdm = np.where(mok[h][None, :, None], pdm, 0.0).transpose(0, 2, 1)
            pd_m_a[h] = pdm[:128].reshape(128, 384)
            pd_m_b[h] = pdm[128:].reshape(PD_M_K - 128, 384)
            w_m[h] = np.where(mok[h][None, :], w_m_np[mc[h]].T, 0.0)

        m = dict(rep)
        pk = lambda a: np.ascontiguousarray(a.transpose(1, 0, 2)).reshape(a.shape[1], -1)
        m.update(sd_s=pk(sd_s), pd_s_a=pk(pd_s_a), pd_s_b=pk(pd_s_b), w_s=pk(w_s),
                 sd_f=pk(sd_f), pd_f=pk(pd_f), wre_f=pk(wre),
                 sd_m=pk(sd_m), pd_m_a=pk(pd_m_a), pd_m_b=pk(pd_m_b), w_m=pk(w_m))
        out = {}
        for k, v in m.items():
            if k in BF16_INPUTS:
                out[k] = np.ascontiguousarray(v.astype(BF16NP))
            elif k in F16_INPUTS:
                out[k] = np.ascontiguousarray(v.astype(F16NP))
            else:
                out[k] = np.ascontiguousarray(v, f32)
        in_maps.append(out)

    return in_maps, vid_all


# ================================================================ device IR

def _build_nc():
    nc = bacc.Bacc("TRN2", target_bir_lowering=False, debug=False,
                   num_devices=NCORES)
    di = {}

    def din(name, shape):
        dt = BF16 if name in BF16_INPUTS else (F16 if name in F16_INPUTS else F32)
        di[name] = nc.dram_tensor(name, list(shape), dt, kind="ExternalInput").ap()

    din("cb32", (128, 763)); din("cb16", (128, 1849))
    din("sd_s", (128, NCH_PLAIN * 1152))
    din("pd_s_a", (128, NCH * 384)); din("pd_s_b", (PD_S_K - 128, NCH * 384))
    din("w_s", (55, NCH * 128))
    din("sd_f", (128, 3 * 1152)); din("pd_f", (PD_F_K, 3 * 384))
    din("wre_f", (11, 3 * 128))
    din("sd_m", (11, 2 * 384)); din("pd_m_a", (128, 2 * 384))
    din("pd_m_b", (PD_M_K - 128, 2 * 384)); din("w_m", (16, 2 * 128))

    out_d = nc.dram_tensor("out", [ROWS, 384], F16, kind="ExternalOutput").ap()
    dbg_d = None
    if DEBUG:
        dbg_d = nc.dram_tensor("dbg", [128, 2048], F32, kind="ExternalOutput").ap()

    with tile.TileContext(nc) as tc:
        _emit(nc, tc, di, out_d, dbg_d)
    nc.compile()
    return nc


def _emit(nc, tc, di, out_d, dbg_d=None):
    par = _fk_forest()
    levels_v = dict(_fk_levels(par, range(0, 60)))            # body+fingers+flame
    # hands: local forest, joints 0..31 (L root 0, R root 16)
    levels_p = {1: [(1, 3, 5, 0, 0), (17, 3, 5, 16, 0)],
                2: [(2, 3, 5, 1, 3), (18, 3, 5, 17, 3)],
                3: [(3, 3, 5, 2, 3), (19, 3, 5, 18, 3)]}
    max_d = max(list(levels_v) + list(levels_p))

    es = ExitStack()
    persist = es.enter_context(tc.tile_pool(name="persist", bufs=1))
    slabs = es.enter_context(tc.tile_pool(name="slabs", bufs=3))
    jpool_cm = tc.tile_pool(name="jpool", bufs=1, space="PSUM")
    jpool = jpool_cm.__enter__()

    V, S, G, T, DMA = nc.vector, nc.scalar, nc.gpsimd, nc.tensor, nc.sync
    P = nc.gpsimd

    def ptile(shape, name, dt=F32):
        return persist.tile(list(shape), dt, tag=name, name=name)

    # ---------------- constants: two packed DMAs -------------------------
    cb32 = persist.tile([128, 763], F32, tag="cb32", name="cb32")
    DMA.dma_start(cb32[:], di["cb32"][:])
    cb16 = persist.tile([128, 1849], BF16, tag="cb16", name="cb16")
    DMA.dma_start(cb16[:], di["cb16"][:])
    c32, c16 = cb32[:], cb16[:]
    ident = c32[:, 0:128]
    aa = c32[:, 128:293]
    aux = c32[:, 293:307]
    jmb = c32[:, 307:355]
    relmb = c32[:, 355:403]
    joffb = c32[:, 403:568]
    joffrelb = c32[:, 568:733]
    betaT_s = c16[:, 0:384]
    betaT_f = c16[:, 384:768]
    jd_s = c16[:, 768:1263]
    jdr_s = c16[:, 1263:1758]
    jd_f = c16[:, 1758:1803]
    jdr_f = c16[:, 1803:1848]
    betam = c16[0:11, 1848:1849]

    # ---------------- staged model tensors (few big DMAs) ----------------
    sd_s_t = persist.tile([128, NCH_PLAIN * 1152], BF16, tag="sd_s", name="sd_s_t")
    pd_a_t = persist.tile([128, NCH * 384], BF16, tag="pd_a", name="pd_a_t")
    pd_b_t = persist.tile([PD_S_K - 128, NCH * 384], BF16, tag="pd_b", name="pd_b_t")
    w_s_t = persist.tile([55, NCH * 128], F16, tag="w_s", name="w_s_t")
    sd_f_t = persist.tile([128, 3 * 1152], BF16, tag="sd_f", name="sd_f_t")
    pd_f_t = persist.tile([PD_F_K, 3 * 384], BF16, tag="pd_f", name="pd_f_t")
    wre_t = persist.tile([11, 384], F16, tag="wre", name="wre_t")
    sd_m_t = persist.tile([11, 768], BF16, tag="sd_m", name="sd_m_t")
    pd_ma_t = persist.tile([128, 768], BF16, tag="pd_ma", name="pd_ma_t")
    pd_mb_t = persist.tile([PD_M_K - 128, 768], BF16, tag="pd_mb", name="pd_mb_t")
    w_m_t = persist.tile([16, 256], F16, tag="w_m", name="w_m_t")
    for h in range(2):
        DMA.dma_start(sd_s_t[:, h * 3456:(h + 1) * 3456],
                      di["sd_s"][:, h * 3456:(h + 1) * 3456])
        DMA.dma_start(pd_a_t[:, h * 2112:(h + 1) * 2112],
                      di["pd_s_a"][:, h * 2112:(h + 1) * 2112])
        DMA.dma_start(pd_b_t[:, h * 2112:(h + 1) * 2112],
                      di["pd_s_b"][:, h * 2112:(h + 1) * 2112])
    DMA.dma_start(sd_f_t[:], di["sd_f"][:])
    DMA.dma_start(pd_f_t[:], di["pd_f"][:])
    DMA.dma_start(sd_m_t[:], di["sd_m"][:])
    DMA.dma_start(pd_ma_t[:], di["pd_m_a"][:])
    DMA.dma_start(pd_mb_t[:], di["pd_m_b"][:])
    DMA.dma_start(w_s_t[:], di["w_s"][:])
    DMA.dma_start(wre_t[:], di["wre_f"][:])
    DMA.dma_start(w_m_t[:], di["w_m"][:])

    # ---------------- joints + rel, batch layout (12 small MMs) ----------
    # out[b, (c,j)] = sum_l betaT[l, b] * jd[l, (c,j)]  (stationary = betaT)
    jp = jpool.tile([128, 2048], F32, tag="jp", name="jp")
    for lk in range(3):
        st, sp = (lk == 0), (lk == 2)
        T.matmul(jp[:, 0:165], betaT_s[:, lk * 128:(lk + 1) * 128],
                 jd_s[:, lk * 165:(lk + 1) * 165], start=st, stop=sp)
        T.matmul(jp[:, 512:677], betaT_s[:, lk * 128:(lk + 1) * 128],
                 jdr_s[:, lk * 165:(lk + 1) * 165], start=st, stop=sp)
        T.matmul(jp[:, 1024:1039], betaT_f[:, lk * 128:(lk + 1) * 128],
                 jd_f[:, lk * 15:(lk + 1) * 15], start=st, stop=sp)
        T.matmul(jp[:, 1536:1551], betaT_f[:, lk * 128:(lk + 1) * 128],
                 jdr_f[:, lk * 15:(lk + 1) * 15], start=st, stop=sp)

    jb = ptile((B, 165), "jb")
    relb = ptile((B, 165), "relb")
    jfb = ptile((B, 15), "jfb")
    relfb = ptile((B, 15), "relfb")
    V.tensor_add(jb[:], jp[:, 0:165], joffb)
    V.tensor_add(relb[:], jp[:, 512:677], joffrelb)
    S.copy(jfb[:], jp[:, 1024:1039])
    S.copy(relfb[:], jp[:, 1536:1551])
    jpool_cm.__exit__(None, None, None)
    acc_cm = tc.tile_pool(name="acc", bufs=2, space="PSUM")
    acc = acc_cm.__enter__()

    def transpose_to(dst_ap, src_ap):
        pp = acc.tile([128, 512], F32, tag="tpose")
        k, n = src_ap.shape[0], src_ap.shape[1]
        T.matmul(pp[:n, :k], src_ap, ident[0:k, 0:k], is_transpose=True,
                 start=True, stop=True)
        S.copy(dst_ap, pp[:n, :k])

    # ---------------- FK static staging (no rodrigues dependency) --------
    NJV = 60
    Tb = ptile((B, NJV * 12), "Tb")
    Ab = ptile((B, NJV * 12), "Ab")
    T4 = Tb[:].rearrange("p (j m n) -> p j m n", m=3, n=4)
    A4 = Ab[:].rearrange("p (j m n) -> p j m n", m=3, n=4)
    Tbh = ptile((B, 32 * 12), "Tbh")
    Abh = ptile((B, 32 * 12), "Abh")
    T4h = Tbh[:].rearrange("p (j m n) -> p j m n", m=3, n=4)
    A4h = Abh[:].rearrange("p (j m n) -> p j m n", m=3, n=4)
    G.memset(Tb[:], 0.0)
    G.memset(Tbh[:], 0.0)
    for j0, n in ((22, 33), (OFF_F, 2)):
        G.memset(Tb[:].rearrange("p (j x) -> p j x", x=12)[:, j0:j0 + n, 0:11:5], 1.0)
    for j0 in (0, 16):
        G.memset(Tbh[:].rearrange("p (j x) -> p j x", x=12)[:, j0:j0 + 1, 0:11:5], 1.0)
    S.copy(T4[:, 0:55, :, 3], relb[:].rearrange("p (c j) -> p j c", c=3))
    S.copy(T4[:, OFF_F:OFF_F + 5, :, 3],
           relfb[:].rearrange("p (c j) -> p j c", c=3))
    for jh in (0, 16):
        P.tensor_copy(T4h[:, jh:jh + 16, :, 3],
                      relmb.rearrange("p (c j) -> p j c", c=3))

    # ---------------- rodrigues ------------------------------------------
    rot = ptile((B, NROT * 9), "rot")
    _rodrigues(nc, aa, rot, ptile)
    rot4 = rot[:].rearrange("p (j x) -> p j x", x=9)

    def pf_make(name, j0, n):
        t = ptile((B, n * 9), name)
        t9 = t[:].rearrange("p (j x) -> p j x", x=9)
        V.tensor_copy(t9, rot4[:, j0:j0 + n, :])
        V.tensor_scalar_add(t9[:, :, 0:9:4], t9[:, :, 0:9:4], -1.0)
        return t

    pf_s = pf_make("pf_s", 1, 21)
    pf_f = pf_make("pf_f", 22, 3)
    pf_m = [pf_make("pf_l", 25, 15), pf_make("pf_r", 40, 15)]

    pfT_s_a = ptile((128, 128), "pfT_s_a", BF16)
    pfT_s_b = ptile((PD_S_K - 128, 128), "pfT_s_b", BF16)
    transpose_to(pfT_s_a[:], pf_s[:, 0:128])
    transpose_to(pfT_s_b[:], pf_s[:, 128:PD_S_K])
    pfT_f = ptile((PD_F_K, 128), "pfT_f", BF16)
    transpose_to(pfT_f[:], pf_f[:, :])
    pfT_m_a = [ptile((128, 128), "pfT_l_a", BF16), ptile((128, 128), "pfT_r_a", BF16)]
    pfT_m_b = [ptile((PD_M_K - 128, 128), "pfT_l_b", BF16),
               ptile((PD_M_K - 128, 128), "pfT_r_b", BF16)]
    for h in range(2):
        transpose_to(pfT_m_a[h][:], pf_m[h][:, 0:128])
        transpose_to(pfT_m_b[h][:], pf_m[h][:, 128:PD_M_K])

    # ---------------- stage A: blend shapes (fp16 v_posed out) -----------
    vp16 = [ptile((128, 384), f"vp{i}", F16) for i in range(NCH)]
    vpf16 = [ptile((128, 384), f"vpf{h}", F16) for h in range(3)]
    vpm16 = [ptile((128, 384), f"vpm{h}", F16) for h in range(2)]

    def stage_a_chunk(i):
        pq = acc.tile([128, 512], F32, tag="vppsum", bufs=4)
        pda = pd_a_t[:, i * 384:(i + 1) * 384]
        pdb = pd_b_t[:, i * 384:(i + 1) * 384]
        if i in CH_PLAIN:
            sdt = sd_s_t[:, i * 1152:(i + 1) * 1152]
            for c3 in range(3):
                for lk in range(3):
                    T.matmul(pq[:, c3 * 128:(c3 + 1) * 128],
                             sdt[:, (c3 * 3 + lk) * 128:(c3 * 3 + lk + 1) * 128],
                             betaT_s[:, lk * 128:(lk + 1) * 128],
                             start=(lk == 0), stop=False)
                T.matmul(pq[:, c3 * 128:(c3 + 1) * 128],
                         pda[:, c3 * 128:(c3 + 1) * 128], pfT_s_a[:],
                         start=False, stop=False)
                T.matmul(pq[:, c3 * 128:(c3 + 1) * 128],
                         pdb[:, c3 * 128:(c3 + 1) * 128], pfT_s_b[:],
                         start=False, stop=True)
        else:
            for c3 in range(3):
                T.matmul(pq[:, c3 * 128:(c3 + 1) * 128],
                         pda[:, c3 * 128:(c3 + 1) * 128], pfT_s_a[:],
                         start=True, stop=False)
                T.matmul(pq[:, c3 * 128:(c3 + 1) * 128],
                         pdb[:, c3 * 128:(c3 + 1) * 128], pfT_s_b[:],
                         start=False, stop=True)
        S.copy(vp16[i][:], pq[:, 0:384])

    def stage_a_flame(h):
        sdt = sd_f_t[:, h * 1152:(h + 1) * 1152]
        pdf = pd_f_t[:, h * 384:(h + 1) * 384]
        pq = big.tile([128, 1536], F32, tag="bigp")
        for c3 in range(3):
            for lk in range(3):
                T.matmul(pq[:, c3 * 128:(c3 + 1) * 128],
                         sdt[:, (c3 * 3 + lk) * 128:(c3 * 3 + lk + 1) * 128],
                         betaT_f[:, lk * 128:(lk + 1) * 128],
                         start=(lk == 0), stop=False)
            T.matmul(pq[:, c3 * 128:(c3 + 1) * 128],
                     pdf[:, c3 * 128:(c3 + 1) * 128], pfT_f[:],
                     start=False, stop=True)
        S.copy(vpf16[h][:], pq[:, 0:384])

    def stage_a_mano(h):
        sdt = sd_m_t[:, h * 384:(h + 1) * 384]
        pq = big.tile([128, 1536], F32, tag="bigp")
        for c3 in range(3):
            T.matmul(pq[:, 384 + c3:385 + c3], sdt[:, c3 * 128:(c3 + 1) * 128],
                     betam, start=True, stop=True)
        pda = pd_ma_t[:, h * 384:(h + 1) * 384]
        pdb = pd_mb_t[:, h * 384:(h + 1) * 384]
        for c3 in range(3):
            T.matmul(pq[:, c3 * 128:(c3 + 1) * 128],
                     pda[:, c3 * 128:(c3 + 1) * 128], pfT_m_a[h][:],
                     start=True, stop=False)
            T.matmul(pq[:, c3 * 128:(c3 + 1) * 128],
                     pdb[:, c3 * 128:(c3 + 1) * 128], pfT_m_b[h][:],
                     start=False, stop=True)
        vshm = ptile((128, 3), f"vshm{h}")
        S.copy(vshm[:], pq[:, 384:387])
        for c3 in range(3):
            S.add(vpm16[h][:, c3 * 128:(c3 + 1) * 128],
                  pq[:, c3 * 128:(c3 + 1) * 128], vshm[:, c3:c3 + 1])

    for i in range(NCH):
        stage_a_chunk(i)

    # ---------------- FK -------------------------------------------------
    # ---------------- FK -------------------------------------------------
    # ---------------- FK -------------------------------------------------
    def rot_to_T(eng, t4v, tj0, rj0, n):
        eng.tensor_copy(t4v[:, tj0:tj0 + n, :, 0:3],
                        rot4[:, rj0:rj0 + n, :].rearrange("p j (m n) -> p j m n", n=3))

    rot_to_T(V, T4, 0, ROT_S0, 22)
    rot_to_T(V, T4, OFF_F + 2, ROT_F0, 3)
    rot_to_T(P, T4h, 1, ROT_L0, 15)
    rot_to_T(P, T4h, 17, ROT_R0, 15)

    V.tensor_copy(A4[:, 0:1], T4[:, 0:1])
    V.tensor_copy(A4[:, OFF_F:OFF_F + 1], T4[:, OFF_F:OFF_F + 1])
    P.tensor_copy(A4h[:, 0:1], T4h[:, 0:1])
    P.tensor_copy(A4h[:, 16:17], T4h[:, 16:17])

    fk_scr_v = ptile((B, 12 * 16), "fk_scr_v")
    fk_scr_p = ptile((B, 12 * 16), "fk_scr_p")

    def fk_run(eng, a4v, t4v, scr_t, run):
        d0, ds, n, p0, ps = run
        sl_d = slice(d0, d0 + (n - 1) * ds + 1, ds) if ds != 1 else slice(d0, d0 + n)
        dst, dT = a4v[:, sl_d], t4v[:, sl_d]
        if ps == 0:
            parw = a4v[:, p0:p0 + 1].broadcast_to([B, n, 3, 4])
        else:
            sl_p = slice(p0, p0 + (n - 1) * ps + 1, ps) if ps != 1 else slice(p0, p0 + n)
            parw = a4v[:, sl_p]
        sc = scr_t[:].rearrange("p (j m n) -> p j m n", m=3, n=4)[:, :n]
        for k in range(3):
            a_k = parw[:, :, :, k:k + 1].broadcast_to([B, n, 3, 4])
            t_k = dT[:, :, k:k + 1, :].broadcast_to([B, n, 3, 4])
            if k == 0:
                eng.tensor_mul(dst, a_k, t_k)
            else:
                eng.tensor_mul(sc, a_k, t_k)
                eng.tensor_add(dst, dst, sc)
        eng.tensor_add(dst[:, :, :, 3], dst[:, :, :, 3], parw[:, :, :, 3])

    for d in range(1, max_d + 1):
        for run in levels_v.get(d, ()):
            fk_run(V, A4, T4, fk_scr_v, run)
        for run in levels_p.get(d, ()):
            fk_run(P, A4h, T4h, fk_scr_p, run)

    # ---- per-batch staging (world translations BEFORE rel-correction) ----
    hm = ptile((B, 16), "hm")
    jb3 = jb[:].rearrange("p (c j) -> p c j", c=3)
    jm3 = jmb.rearrange("p (c j) -> p c j", c=3)
    bias9 = ptile((B, 9), "bias9")
    V.tensor_add(hm[:, 0:3], jb3[:, :, 23], jb3[:, :, 24])
    V.tensor_add(hm[:, 3:6], A4[:, OFF_F + 3, :, 3], A4[:, OFF_F + 4, :, 3])
    V.tensor_sub(hm[:, 6:9], hm[:, 0:3], hm[:, 3:6])
    V.tensor_scalar_mul(hm[:, 6:9], hm[:, 6:9], 0.5)
    V.tensor_add(bias9[:, 0:3], hm[:, 6:9], aux[:, 5:8])
    V.tensor_sub(hm[:, 9:12], aux[:, 8:11], jm3[:, :, 0])
    V.tensor_sub(bias9[:, 3:4], jb3[:, 0:1, 20], hm[:, 9:10])
    V.tensor_add(bias9[:, 4:6], hm[:, 10:12], jb3[:, 1:3, 20])
    V.tensor_sub(hm[:, 12:15], aux[:, 11:14], jm3[:, :, 0])
    V.tensor_add(bias9[:, 6:9], hm[:, 12:15], jb3[:, :, 21])
    epp = ptile((B, 2), "epp")
    V.tensor_mul(epp[:], aux[:, 3:5], aux[:, 0:1].broadcast_to([B, 2]))

    # ---- scale folding (commutes with corr; must precede rot fills) ------
    V.tensor_scalar_mul(Ab[:, OFF_F * 12:(OFF_F + 5) * 12],
                        Ab[:, OFF_F * 12:(OFF_F + 5) * 12], aux[:, 0:1])
    negls = ptile((B, 1), "negls")
    P.tensor_scalar_mul(negls[:], aux[:, 1:2], -1.0)
    AL = A4h[:, 0:16]
    P.tensor_mul(AL[:, :, 0, :], AL[:, :, 0, :],
                 negls[:, 0:1].unsqueeze(2).broadcast_to([B, 16, 4]))
    P.tensor_mul(AL[:, :, 1:3, :], AL[:, :, 1:3, :],
                 aux[:, 1:2].unsqueeze(2).unsqueeze(3).broadcast_to([B, 16, 2, 4]))
    ARr = A4h[:, 16:32]
    P.tensor_mul(ARr[:, :, :, :], ARr[:, :, :, :],
                 aux[:, 2:3].unsqueeze(2).unsqueeze(3).broadcast_to([B, 16, 3, 4]))

    # ---- rhs rotation blocks (n4=0..2): independent of corr --------------
    def rhs_fill(rhs_t, a4v, j0, nj, col0, n4):
        pp = acc.tile([128, 512], F32, tag="tpose")
        for m3 in range(3):
            T.matmul(pp[0:nj, m3 * 128:(m3 + 1) * 128],
                     a4v[:, j0:j0 + nj, m3, n4], ident,
                     is_transpose=True, start=True, stop=True)
        S.copy(rhs_t[0:nj, col0:col0 + 384], pp[0:nj, 0:384])

    rhs_s = ptile((55, 1536), "rhs_s", F16)
    rhs_f = ptile((11, 1536), "rhs_f", F16)
    G.memset(rhs_f[:], 0.0)
    rhs_m = [ptile((16, 1536), "rhs_l", F16), ptile((16, 1536), "rhs_r", F16)]
    for n4 in range(3):
        rhs_fill(rhs_s, A4, 0, 55, n4 * 384, n4)
        rhs_fill(rhs_f, A4, OFF_F, 5, n4 * 384, n4)
        for h, off in ((0, 0), (1, 16)):
            rhs_fill(rhs_m[h], A4h, off, 16, n4 * 384, n4)

    # ---- A_rel: translation -= R_world @ J (scaled R: commutes) ----------
    corr_v_a = ptile((B, 55 * 3), "corr_v_a")
    corr_v_b = ptile((B, 55 * 3), "corr_v_b")
    corr_p_a = ptile((B, 16 * 3), "corr_p_a")
    corr_p_b = ptile((B, 16 * 3), "corr_p_b")

    def corr(eng, a4v, ct_t, ct2_t, j0, nj, jsrc, joff_):
        ct = ct_t[:].rearrange("p (j m) -> p j m", m=3)[:, 0:nj]
        ct2 = ct2_t[:].rearrange("p (j m) -> p j m", m=3)[:, 0:nj]
        js = jsrc.rearrange("p (c j) -> p c j", c=3)
        for k in range(3):
            a_k = a4v[:, j0:j0 + nj, :, k]
            j_k = js[:, k, joff_:joff_ + nj].unsqueeze(2).broadcast_to([B, nj, 3])
            if k == 0:
                eng.tensor_mul(ct, a_k, j_k)
            else:
                eng.tensor_mul(ct2, a_k, j_k)
                eng.tensor_add(ct, ct, ct2)
        eng.tensor_sub(a4v[:, j0:j0 + nj, :, 3], a4v[:, j0:j0 + nj, :, 3], ct)

    corr(V, A4, corr_v_a, corr_v_b, 0, 55, jb[:], 0)
    corr(V, A4, corr_v_a, corr_v_b, OFF_F, 5, jfb[:], 0)
    corr(P, A4h, corr_p_a, corr_p_b, 0, 16, jmb, 0)
    corr(P, A4h, corr_p_a, corr_p_b, 16, 16, jmb, 0)

    # ---- bias fold into corrected translations ---------------------------
    V.tensor_add(A4[:, OFF_F:OFF_F + 5, :, 3], A4[:, OFF_F:OFF_F + 5, :, 3],
                 bias9[:, 0:3].unsqueeze(1).broadcast_to([B, 5, 3]))
    P.tensor_add(AL[:, :, :, 3], AL[:, :, :, 3],
                 bias9[:, 3:6].unsqueeze(1).broadcast_to([B, 16, 3]))
    P.tensor_add(ARr[:, :, :, 3], ARr[:, :, :, 3],
                 bias9[:, 6:9].unsqueeze(1).broadcast_to([B, 16, 3]))

    # ---- rhs translation blocks (n4=3) + eyelid rows ---------------------
    rhs_fill(rhs_s, A4, 0, 55, 3 * 384, 3)
    rhs_fill(rhs_f, A4, OFF_F, 5, 3 * 384, 3)
    for h, off in ((0, 0), (1, 16)):
        rhs_fill(rhs_m[h], A4h, off, 16, 3 * 384, 3)
    epT = ptile((2, 128), "epT", F16)
    transpose_to(epT[:], epp[:, :])
    for m3 in range(3):
        DMA.dma_start(rhs_f[5 + m3:6 + m3, (9 + m3) * 128:(10 + m3) * 128],
                      epT[1:2, :])
        DMA.dma_start(rhs_f[8 + m3:9 + m3, (9 + m3) * 128:(10 + m3) * 128],
                      epT[0:1, :])

    acc_cm.__exit__(None, None, None)
    big_cm = tc.tile_pool(name="big", bufs=2, space="PSUM")
    big = big_cm.__enter__()

    # ---------------- skinning per chunk ---------------------------------
    # All t_apply on V (fp16, 2x DVE); psum->fp16 copies split between
    # Scalar and Pool to keep both off the critical path.
    def t_apply_v(dst_ap, tp16_ap, x16_ap, pr_ap):
        tp4 = tp16_ap.rearrange("p (n m b) -> p n m b", m=3, b=128)
        d3 = dst_ap.rearrange("p (m b) -> p m b", b=128)
        x4 = x16_ap.rearrange("p (c b) -> p c b", b=128).unsqueeze(2)
        pr = pr_ap.rearrange("p (n m b) -> p n m b", m=3, b=128)
        V.tensor_mul(pr, tp4[:, 0:3], x4.broadcast_to([128, 3, 3, 128]))
        V.tensor_add(d3, pr[:, 0], pr[:, 1])
        V.tensor_add(d3, d3, pr[:, 2])
        V.tensor_add(d3, d3, tp4[:, 3])

    pr16 = ptile((128, 1152), "pr16", F16)

    def skin_chunk(i):
        if CH_HEAD0 <= i < CH_HEAD0 + 3:
            h = i - CH_HEAD0
            wt = wre_t[:, h * 128:(h + 1) * 128]
            tpm = big.tile([128, 1536], F32, tag="bigp")
            for g in range(3):
                T.matmul(tpm[:, g * 512:(g + 1) * 512], wt,
                         rhs_f[0:11, g * 512:(g + 1) * 512], start=True, stop=True)
            tp16 = slabs.tile((128, 1536), F16, tag="tp16", bufs=3)
            S.copy(tp16[:], tpm[:])
            hv = slabs.tile((128, 384), F16, tag="hv", bufs=2)
            t_apply_v(hv[:], tp16[:], vpf16[h][:], pr16[:])
            V.tensor_add(vp16[i][:], vp16[i][:], hv[:])
        elif i in (CH_HL, CH_HR):
            h = i - CH_HL
            wt = w_m_t[:, h * 128:(h + 1) * 128]
            tpm = big.tile([128, 1536], F32, tag="bigp")
            for g in range(3):
                T.matmul(tpm[:, g * 512:(g + 1) * 512], wt,
                         rhs_m[h][:, g * 512:(g + 1) * 512], start=True, stop=True)
            tp16 = slabs.tile((128, 1536), F16, tag="tp16", bufs=3)
            S.copy(tp16[:], tpm[:])
            hv = slabs.tile((128, 384), F16, tag="hv", bufs=2)
            t_apply_v(hv[:], tp16[:], vpm16[h][:], pr16[:])
            V.tensor_add(vp16[i][:], vp16[i][:], hv[:])

        wt = w_s_t[:, i * 128:(i + 1) * 128]
        tps = big.tile([128, 1536], F32, tag="bigp")
        for g in range(3):
            T.matmul(tps[:, g * 512:(g + 1) * 512], wt,
                     rhs_s[:, g * 512:(g + 1) * 512], start=True, stop=True)
        ot = slabs.tile((128, 384), F16, tag="outt", bufs=3)
        tp16 = slabs.tile((128, 1536), F16, tag="tp16", bufs=3)
        S.copy(tp16[:], tps[:])
        t_apply_v(ot[:], tp16[:], vp16[i][:], pr16[:])
        DMA.dma_start(out_d[i * 128:(i + 1) * 128, :], ot[:])

    for i in range(NCH_PLAIN):
        skin_chunk(i)
    for h in range(3):
        stage_a_flame(h)
    for h in range(2):
        stage_a_mano(h)
    for i in range(NCH_PLAIN, NCH):
        skin_chunk(i)

    big_cm.__exit__(None, None, None)
    es.close()


def _rodrigues(nc, aa, rot, ptile):
    V, S = nc.vector, nc.scalar
    J = NROT
    aa3 = aa[:].rearrange("p (j k) -> p j k", k=3)
    sq = ptile((B, J), "rg_sq")
    tmp = ptile((B, J), "rg_tmp")
    V.tensor_mul(sq[:], aa3[:, :, 0], aa3[:, :, 0])
    V.tensor_mul(tmp[:], aa3[:, :, 1], aa3[:, :, 1])
    V.tensor_add(sq[:], sq[:], tmp[:])
    V.tensor_mul(tmp[:], aa3[:, :, 2], aa3[:, :, 2])
    V.tensor_add(sq[:], sq[:], tmp[:])
    eps_t = ptile((B, 1), "rg_eps")
    nc.gpsimd.memset(eps_t[:], 1e-8)
    hpi_t = ptile((B, 1), "rg_hpi")
    nc.gpsimd.memset(hpi_t[:], float(np.pi / 2))
    zero_t = ptile((B, 1), "rg_zero")
    nc.gpsimd.memset(zero_t[:], 0.0)
    ang = ptile((B, J), "rg_ang")
    S.activation(ang[:], sq[:], AF.Sqrt, bias=eps_t[:])
    inv = ptile((B, J), "rg_inv")
    V.reciprocal(inv[:], ang[:])
    sn = ptile((B, J), "rg_sin")
    co = ptile((B, J), "rg_cos")
    S.activation(sn[:], ang[:], AF.Sin, bias=zero_t[:])
    S.activation(co[:], ang[:], AF.Sin, bias=hpi_t[:])
    nv = ptile((B, 3 * J), "rg_n")
    n3 = nv[:].rearrange("p (j k) -> p j k", k=3)
    V.tensor_mul(n3, aa3, inv[:].unsqueeze(2).broadcast_to([B, J, 3]))
    u = ptile((B, J), "rg_u")
    V.tensor_scalar(u[:], co[:], -1.0, 1.0, ALU.mult, ALU.add)
    un = ptile((B, 3 * J), "rg_un")
    un3 = un[:].rearrange("p (j k) -> p j k", k=3)
    V.tensor_mul(un3, n3, u[:].unsqueeze(2).broadcast_to([B, J, 3]))
    q = ptile((B, 3 * J), "rg_q")
    q3 = q[:].rearrange("p (j k) -> p j k", k=3)
    V.tensor_mul(q3, un3, n3)
    d = ptile((B, J), "rg_d")
    V.tensor_add(d[:], q3[:, :, 0], q3[:, :, 1])
    V.tensor_add(d[:], d[:], q3[:, :, 2])
    dd = ptile((B, J), "rg_dd")
    V.tensor_scalar(dd[:], d[:], -1.0, 1.0, ALU.mult, ALU.add)
    snv = ptile((B, 3 * J), "rg_snv")
    s3 = snv[:].rearrange("p (j k) -> p j k", k=3)
    V.tensor_mul(s3, n3, sn[:].unsqueeze(2).broadcast_to([B, J, 3]))
    r4 = rot[:].rearrange("p (j m n) -> p j m n", m=3, n=3)
    for m in range(3):
        V.tensor_add(r4[:, :, m, m], q3[:, :, m], dd[:])
    p = ptile((B, J), "rg_p")
    V.tensor_mul(p[:], un3[:, :, 0], n3[:, :, 1])
    V.tensor_sub(r4[:, :, 0, 1], p[:], s3[:, :, 2])
    V.tensor_add(r4[:, :, 1, 0], p[:], s3[:, :, 2])
    V.tensor_mul(p[:], un3[:, :, 0], n3[:, :, 2])
    V.tensor_add(r4[:, :, 0, 2], p[:], s3[:, :, 1])
    V.tensor_sub(r4[:, :, 2, 0], p[:], s3[:, :, 1])
    V.tensor_mul(p[:], un3[:, :, 1], n3[:, :, 2])
    V.tensor_sub(r4[:, :, 1, 2], p[:], s3[:, :, 0])
    V.tensor_add(r4[:, :, 2, 1], p[:], s3[:, :, 0])


# ================================================================ entry

_CACHED = {}
PROFILE = False
DEBUG = False


def _get_nc():
    if "nc" not in _CACHED:
        _CACHED["nc"] = _build_nc()
    return _CACHED["nc"]


def kernel(**inputs):
    in_maps, vid_all = _host_prep(inputs)
    nc = _get_nc()
    res = run_bass_kernel_spmd(nc, in_maps, core_ids=list(range(NCORES)),
                               trace=PROFILE)
    _CACHED["last_res"] = res
    out = np.zeros((B, VS, 3), np.float32)
    for c in range(NCORES):
        o = np.asarray(res.results[c]["out"]).astype(np.float32).reshape(ROWS, 3, B)
        vok = vid_all[c] >= 0
        out[:, vid_all[c][vok], :] = o[vok].transpose(2, 0, 1)
    return out

